# revision 1
# baseline (speedup 1.0000x reference)
"""Multi-head attention (B=4, T=2048, D=1024, H=16) on 8 Trainium2 NeuronCores.

Sharding: core = (batch, head-group): b = core // 2, g = core % 2.
Each core computes heads [g*8, g*8+8) of batch b:
  - Q/K projections into transposed layout qT/kT = W_g @ x_b.T  [512, 2048]
  - V projection in natural layout [2048, 512] (+ ones column per head)
  - scores computed transposed: S.T tile = K_h @ Q_h.T on the PE; head pairs
    (2i, 2i+1) sit at partition bases 0/64 so their score matmuls run
    concurrently in distinct PE row groups
  - exp fused on ScalarE over two-bank PSUM groups (FD=1024), scale=1/sqrt(64),
    no max subtraction (logits ~N(0,1))
  - PV with lhsT = [V_h | 1] gives O.T[64, tq] and the softmax row-sums in row 64
  - normalize via reciprocal (reshaped across partitions) + partition-broadcast
  - partial output projection yT_g = Wo[:, g].T-contraction  [1024, 2048]
Host: y[b] = (yT_part[2b] + yT_part[2b+1]).T + bo.

The emission order software-pipelines the engines (head-pair-outer,
query-chunk-inner): Q/K m-tile prefetch is spread one n-chunk per window,
the V projection sits in pair 0, and the output projection trails per chunk
in pair 3, so the PE fills the gaps while ScalarE streams exp().

Self-contained: hardcodes all shapes; requires only concourse (bass) + numpy.
"""

import numpy as np

B, T, D = 4, 2048, 1024
H, HD = 16, 64
HG, DG = 8, 512          # heads / feature columns per core
NCORES = 8
P = 128
KD = D // P              # 8  k-tiles over model dim
MQ = DG // P             # 4  partition tiles of qT/kT/oT (one per head pair)
TK = T // P              # 16 key tiles
TQC = 512                # query-chunk (= one fp32 PSUM bank)
NC2 = T // TQC           # 4  query chunks
VW = HD + 1              # V columns per head incl. ones column
SCALE = 0.125            # 1/sqrt(HD)

_CACHE: dict = {}


def _emit(tc, aps, dbg=None, reps=1):
    import concourse.bass as bass  # noqa: F401
    from concourse import mybir

    nc = tc.nc
    dt = mybir.dt
    f32, bf16 = dt.float32, dt.bfloat16
    AF = mybir.ActivationFunctionType
    xT, wq, wk, wv, wo, bq, bk, bv, yT = (
        aps["xT"], aps["wq"], aps["wk"], aps["wv"], aps["wo"],
        aps["bq"], aps["bk"], aps["bv"], aps["yT"],
    )

    from contextlib import ExitStack

    with ExitStack() as ctx:
        const = ctx.enter_context(tc.tile_pool(name="const", bufs=1))
        persist = ctx.enter_context(tc.tile_pool(name="persist", bufs=1))
        xw = ctx.enter_context(tc.tile_pool(name="xw", bufs=1))
        ptp = ctx.enter_context(tc.tile_pool(name="ptp", bufs=4))
        pvsb = ctx.enter_context(tc.tile_pool(name="pvsb", bufs=2))
        yop = ctx.enter_context(tc.tile_pool(name="yop", bufs=3))
        nrm = ctx.enter_context(tc.tile_pool(name="nrm", bufs=2))
        scps = ctx.enter_context(tc.tile_pool(name="scps", bufs=2, space="PSUM"))
        qkvps = ctx.enter_context(tc.tile_pool(name="qkvps", bufs=2, space="PSUM"))
        pvps = ctx.enter_context(tc.tile_pool(name="pvps", bufs=2, space="PSUM"))

        # ---- persistent SBUF ----
        q_sb = persist.tile([P, MQ, T], bf16)
        k_sb = persist.tile([P, MQ, T], bf16)
        v_sb = persist.tile([P, TK, HG * VW], bf16)
        o_sb = persist.tile([P, MQ, T], bf16)
        v4d = v_sb.rearrange("p t (h c) -> p t h c", h=HG)
        nc.vector.memset(v4d[:, :, :, HD : HD + 1], 1.0)

        # ---- input DMAs: x on the SP queue, weights on the ACT queue so the
        # first q/k projections are not serialized behind the weight loads
        x_sb = xw.tile([P, KD, T], bf16)
        wq_sb = xw.tile([P, KD, DG], bf16)
        wk_sb = xw.tile([P, KD, DG], bf16)
        bq_sb = const.tile([P, MQ], f32)
        nc.sync.dma_start(out=bq_sb, in_=bq)
        bk_sb = const.tile([P, MQ], f32)
        nc.sync.dma_start(out=bk_sb, in_=bk)
        # wk leads the ACT queue (first PE op is its LDWEIGHTS), x is split
        # across both queues, wq follows the even x half on SP
        for ki in range(KD):
            nc.scalar.dma_start(out=wk_sb[:, ki], in_=wk[:, ki])
        for ki in range(0, KD, 2):
            nc.sync.dma_start(out=x_sb[:, ki], in_=xT[:, ki])
        for ki in range(1, KD, 2):
            nc.scalar.dma_start(out=x_sb[:, ki], in_=xT[:, ki])
        for ki in range(KD):
            nc.sync.dma_start(out=wq_sb[:, ki], in_=wq[:, ki])
        wv_sb = xw.tile([P, KD, DG], bf16)
        nc.scalar.dma_start(out=wv_sb, in_=wv)
        bv_sb = xw.tile([1, DG], bf16)
        nc.scalar.dma_start(out=bv_sb, in_=bv)
        ones_sb = xw.tile([1, P], bf16)
        nc.vector.memset(ones_sb, 1.0)
        ones_f32 = xw.tile([1, HD], f32)
        nc.vector.memset(ones_f32, 1.0)
        wo_sb = const.tile([P, MQ, D], bf16)
        nc.scalar.dma_start(out=wo_sb, in_=wo)

        def emit_qk_part(mt, n, which="kq"):
            """One T-chunk (n) of the q and/or k projection for m-tile mt."""
            sel = {
                "k": ((wk_sb, bk_sb, k_sb),),
                "q": ((wq_sb, bq_sb, q_sb),),
                "kq": ((wk_sb, bk_sb, k_sb), (wq_sb, bq_sb, q_sb)),
            }[which]
            # accumulate in x-arrival order (even k-tiles land first on SP)
            ki_order = list(range(0, KD, 2)) + list(range(1, KD, 2))
            for w_sb, b_col, dst in sel:
                ps = qkvps.tile([P, TQC], f32, tag="qkv", name="ps_qkv")
                for idx, ki in enumerate(ki_order):
                    nc.tensor.matmul(
                        ps,
                        w_sb[:, ki, mt * P : (mt + 1) * P],
                        x_sb[:, ki, n * TQC : (n + 1) * TQC],
                        start=(idx == 0),
                        stop=(idx == KD - 1),
                    )
                nc.vector.tensor_scalar_add(
                    dst[:, mt, n * TQC : (n + 1) * TQC], ps, b_col[:, mt : mt + 1]
                )

        def emit_v():
            for t in range(TK):
                ps = qkvps.tile([P, DG], f32, tag="qkv", name="ps_v")
                for ki in range(KD):
                    nc.tensor.matmul(
                        ps,
                        x_sb[:, ki, t * P : (t + 1) * P],
                        wv_sb[:, ki, :],
                        start=(ki == 0),
                        stop=False,
                    )
                nc.tensor.matmul(ps, ones_sb, bv_sb, start=False, stop=True)
                nc.vector.tensor_copy(
                    v4d[:, t, :, 0:HD], ps.rearrange("p (h c) -> p h c", h=HG)
                )

        def scores_exp_pair(p, c, pts):
            """Packed scores for heads (2p, 2p+1): concurrent PE row groups;
            exp over two-bank groups (FD = 2*TQC)."""
            tq0 = c * TQC
            for tkp in range(TK // 2):
                scs = [
                    scps.tile([P, 2, TQC], f32, tag="sc", name="sc0"),
                    scps.tile([P, 2, TQC], f32, tag="sc", name="sc1"),
                ]
                for u in range(2):
                    tk = 2 * tkp + u
                    for i in range(2):
                        hb = i * HD
                        nc.tensor.matmul(
                            scs[i][:, u, :],
                            k_sb[hb : hb + HD, p, tk * P : (tk + 1) * P],
                            q_sb[hb : hb + HD, p, tq0 : tq0 + TQC],
                            start=True,
                            stop=True,
                        )
                for i in range(2):
                    nc.scalar.activation(
                        pts[i][:, 2 * tkp : 2 * tkp + 2, :], scs[i], AF.Exp, scale=SCALE
                    )

        def pv_norm(p, c, i, pt):
            """PV + row-sum + normalize for head h = 2p + i."""
            h = 2 * p + i
            hb = i * HD
            tq0 = c * TQC
            pv = pvps.tile([VW, TQC], f32, name="pv")
            for tk in range(TK):
                nc.tensor.matmul(
                    pv,
                    v_sb[:, tk, h * VW : (h + 1) * VW],
                    pt[:, tk, :],
                    start=(tk == 0),
                    stop=(tk == TK - 1),
                )
            ps_o = pvsb.tile([VW, TQC], f32, name="ps_o")
            nc.vector.tensor_copy(ps_o, pv)
            rc = nrm.tile([1, TQC], f32, name="rc")
            nc.vector.reciprocal(rc, ps_o[HD : HD + 1, :])
            norm_pend.append((p, c, i, ps_o, rc))
            if dbg is not None and c == 0 and h == 0:
                nc.sync.dma_start(out=dbg["pt"], in_=pt)
                nc.sync.dma_start(out=dbg["pv"], in_=ps_o)
                nc.sync.dma_start(out=dbg["rc"], in_=rc)

        norm_pend = []

        def flush_norm():
            """Broadcast 1/rowsum across partitions with a K=1 ones matmul
            (PE), then multiply. Deferred one window behind the PV so the PE
            never waits on the reciprocal."""
            while norm_pend:
                p, c, i, ps_o, rc = norm_pend.pop(0)
                hb = i * HD
                tq0 = c * TQC
                bc = pvps.tile([HD, TQC], f32, tag="pv", name="bc")
                nc.tensor.matmul(bc, ones_f32, rc, start=True, stop=True)
                nc.vector.tensor_mul(
                    o_sb[hb : hb + HD, p, tq0 : tq0 + TQC], ps_o[0:HD, :], bc
                )

        def emit_oproj(c, alt_pool=False):
            tq0 = c * TQC
            yr = yT
            for j in range(D // P):
                # the final chunk runs after attention: the pv slots are idle,
                # so alternate pools for a 4-slot psum pipeline
                if alt_pool and j % 2 == 1:
                    ys = pvps.tile([P, TQC], f32, tag="pv", name="ys")
                else:
                    ys = qkvps.tile([P, TQC], f32, tag="qkv", name="ys")
                for ki in range(MQ):
                    nc.tensor.matmul(
                        ys,
                        wo_sb[:, ki, j * P : (j + 1) * P],
                        o_sb[:, ki, tq0 : tq0 + TQC],
                        start=(ki == 0),
                        stop=(ki == MQ - 1),
                    )
                yo = yop.tile([P, TQC], f32, name="yo")
                nc.vector.tensor_copy(yo, ys)
                nc.sync.dma_start(out=yr[:, j, tq0 : tq0 + TQC], in_=yo)

        # ---- schedule: pair-outer, chunk-inner ----
        if reps > 1:
            loop_cm = tc.For_i(0, reps, 1)
            loop_cm.__enter__()

        for n in range(NC2):
            emit_qk_part(0, n, "k")
        emit_qk_part(0, 0, "q")

        for p in range(MQ):
            for c in range(NC2):
                pts = [
                    ptp.tile([P, TK, TQC], bf16, tag="pt", name="pt0"),
                    ptp.tile([P, TK, TQC], bf16, tag="pt", name="pt1"),
                ]
                scores_exp_pair(p, c, pts)
                flush_norm()
                if p == 0 and c == 0:
                    emit_v()
                if p == 0 and c < NC2 - 1:
                    emit_qk_part(0, c + 1, "q")
                if p < MQ - 1:
                    emit_qk_part(p + 1, c, "kq")
                if p == MQ - 1 and c > 0:
                    emit_oproj(c - 1)
                pv_norm(p, c, 0, pts[0])
                pv_norm(p, c, 1, pts[1])
        flush_norm()
        emit_oproj(NC2 - 1, alt_pool=True)

        if reps > 1:
            loop_cm.__exit__(None, None, None)

        if dbg is not None:
            nc.sync.dma_start(out=dbg["q"], in_=q_sb)
            nc.sync.dma_start(out=dbg["k"], in_=k_sb)
            nc.sync.dma_start(out=dbg["v"], in_=v_sb)
            nc.sync.dma_start(out=dbg["o"], in_=o_sb)


def _build(debug=False, reps=1):
    import concourse.tile as tile
    from concourse import bacc, mybir

    dt = mybir.dt
    f32, bf16 = dt.float32, dt.bfloat16

    nc = bacc.Bacc("TRN2", target_bir_lowering=False, debug=False)
    # inputs are host-preswizzled into partition-major layouts so every DMA
    # descriptor is a fat contiguous run
    aps = {
        "xT": nc.dram_tensor("xT", [P, KD, T], bf16, kind="ExternalInput").ap(),
        "wq": nc.dram_tensor("wq", [P, KD, DG], bf16, kind="ExternalInput").ap(),
        "wk": nc.dram_tensor("wk", [P, KD, DG], bf16, kind="ExternalInput").ap(),
        "wv": nc.dram_tensor("wv", [P, KD, DG], bf16, kind="ExternalInput").ap(),
        "wo": nc.dram_tensor("wo", [P, MQ, D], bf16, kind="ExternalInput").ap(),
        "bq": nc.dram_tensor("bq", [P, MQ], f32, kind="ExternalInput").ap(),
        "bk": nc.dram_tensor("bk", [P, MQ], f32, kind="ExternalInput").ap(),
        "bv": nc.dram_tensor("bv", [1, DG], bf16, kind="ExternalInput").ap(),
        "yT": nc.dram_tensor("yT", [P, D // P, T], f32, kind="ExternalOutput").ap(),
    }

    dbg = None
    if debug:
        dbg = {
            "q": nc.dram_tensor("dbg_q", [P, MQ, T], bf16, kind="ExternalOutput").ap(),
            "k": nc.dram_tensor("dbg_k", [P, MQ, T], bf16, kind="ExternalOutput").ap(),
            "v": nc.dram_tensor(
                "dbg_v", [P, TK, HG * VW], bf16, kind="ExternalOutput"
            ).ap(),
            "o": nc.dram_tensor("dbg_o", [P, MQ, T], bf16, kind="ExternalOutput").ap(),
            "pt": nc.dram_tensor(
                "dbg_pt", [P, TK, TQC], bf16, kind="ExternalOutput"
            ).ap(),
            "pv": nc.dram_tensor("dbg_pv", [VW, TQC], f32, kind="ExternalOutput").ap(),
            "rc": nc.dram_tensor("dbg_rc", [1, TQC], f32, kind="ExternalOutput").ap(),
        }

    with tile.TileContext(nc) as tc:
        _emit(tc, aps, dbg, reps=reps)
    nc.compile()
    return nc


def _get_nc():
    if "nc" not in _CACHE:
        _CACHE["nc"] = _build()
    return _CACHE["nc"]


def _shard_inputs(x, Wq, bq, Wk, bk, Wv, bv, Wo, bo):
    import ml_dtypes

    bf16 = ml_dtypes.bfloat16
    f32 = np.float32

    def c(a, dtype):
        return np.ascontiguousarray(a).astype(dtype)

    def kp(a, kt):  # [kt*P, F] -> [P, kt, F] partition-major swizzle
        return a.reshape(kt, P, a.shape[-1]).transpose(1, 0, 2)

    in_maps = []
    for core in range(NCORES):
        b, g = core // 2, core % 2
        hs = g * DG
        in_maps.append(
            {
                "xT": c(kp(x[b].T, KD), bf16),
                "wq": c(kp(Wq[hs : hs + DG, :].T, KD), bf16),
                "wk": c(kp(Wk[hs : hs + DG, :].T, KD), bf16),
                "wv": c(kp(Wv[hs : hs + DG, :].T, KD), bf16),
                "wo": c(kp(Wo[:, hs : hs + DG].T, MQ), bf16),
                "bq": c(bq[hs : hs + DG].reshape(MQ, P).T, f32),
                "bk": c(bk[hs : hs + DG].reshape(MQ, P).T, f32),
                "bv": c(bv[hs : hs + DG].reshape(1, DG), bf16),
            }
        )
    return in_maps


def _run(inputs, trace=False):
    from concourse import bass_utils

    nc = _get_nc()
    np_in = {k: np.asarray(v) for k, v in inputs.items()}
    in_maps = _shard_inputs(**np_in)
    res = bass_utils.run_bass_kernel_spmd(
        nc, in_maps, core_ids=list(range(NCORES)), trace=trace
    )
    bo = np_in["bo"].astype(np.float32)
    y = np.empty((B, T, D), dtype=np.float32)
    for b in range(B):
        acc = res.results[2 * b]["yT"] + res.results[2 * b + 1]["yT"]  # [P, D/P, T]
        y[b] = acc.transpose(1, 0, 2).reshape(D, T).T + bo
    return y, res


def kernel(**inputs):
    y, _ = _run(inputs)
    return y



# revision 3
# speedup vs baseline: 1.1374x; 1.1374x over previous
"""Multi-head attention (B=4, T=2048, D=1024, H=16) on 8 Trainium2 NeuronCores.

Sharding: core = (batch, head-group): b = core // 2, g = core % 2.
Each core computes heads [g*8, g*8+8) of batch b:
  - Q/K projections into transposed layout qT/kT = W_g @ x_b.T  [512, 2048]
  - V projection in natural layout [2048, 512] plus a ones column per head
  - scores computed transposed: S.T tile = K_h @ Q_h.T on the PE; exp fused
    on ScalarE over two-bank PSUM groups (FD=1024), scale=1/sqrt(64),
    no max subtraction (logits ~N(0,1))
  - PV transposed: out[128q, 65] = pT-slice[128k,128q].T @ [V_h|1][128k,65]
    so the matmul streams only 65 columns per k-tile (cost model charges
    N=out-free-size) and the softmax row-sum lands lane-wise in column 64
  - normalize is a per-partition reciprocal + tensor_scalar_mul on DVE
  - o is transposed back with PE transpose-matmuls for the output projection
  - partial output projection yT_g = Wo[:, g].T-contraction  [1024, 2048]
Host: y[b] = (yT_part[2b] + yT_part[2b+1]).T + bo + bv @ Wo.T
(softmax rows sum to one, so the V bias contributes exactly bv @ Wo.T).

The emission interleaves PE filler work (PV of the previous window, Q/K
prefetch, V projection, output projection) between score/exp groups so the
PE never waits on ScalarE's exp stream.

Self-contained: hardcodes all shapes; requires only concourse (bass) + numpy.
"""

import numpy as np

B, T, D = 4, 2048, 1024
H, HD = 16, 64
HG, DG = 8, 512          # heads / feature columns per core
NCORES = 8
P = 128
KD = D // P              # 8  k-tiles over model dim
MQ = DG // P             # 4  partition tiles of qT/kT/oT (one per head pair)
TK = T // P              # 16 key tiles
TQC = 512                # query-chunk (= one fp32 PSUM bank)
NC2 = T // TQC           # 4  query chunks
VW = HD + 1              # V columns per head incl. ones column
SCALE = 0.125            # 1/sqrt(HD)

_CACHE: dict = {}


def _emit(tc, aps, reps=1):
    import concourse.bass as bass  # noqa: F401
    from concourse import masks, mybir

    nc = tc.nc
    dt = mybir.dt
    f32, bf16 = dt.float32, dt.bfloat16
    AF = mybir.ActivationFunctionType
    xT, wq, wk, wv, wo, bq, bk, yT = (
        aps["xT"], aps["wq"], aps["wk"], aps["wv"], aps["wo"],
        aps["bq"], aps["bk"], aps["yT"],
    )

    from contextlib import ExitStack

    with ExitStack() as ctx:
        const = ctx.enter_context(tc.tile_pool(name="const", bufs=1))
        persist = ctx.enter_context(tc.tile_pool(name="persist", bufs=1))
        xw = ctx.enter_context(tc.tile_pool(name="xw", bufs=1))
        ptp = ctx.enter_context(tc.tile_pool(name="ptp", bufs=4))
        onp = ctx.enter_context(tc.tile_pool(name="onp", bufs=2))
        yop = ctx.enter_context(tc.tile_pool(name="yop", bufs=3))
        nrm = ctx.enter_context(tc.tile_pool(name="nrm", bufs=3))
        scps = ctx.enter_context(tc.tile_pool(name="scps", bufs=2, space="PSUM"))
        qkvps = ctx.enter_context(tc.tile_pool(name="qkvps", bufs=2, space="PSUM"))
        pvtp = ctx.enter_context(tc.tile_pool(name="pvtp", bufs=2, space="PSUM"))

        # ---- persistent SBUF ----
        q_sb = persist.tile([P, MQ, T], bf16)
        k_sb = persist.tile([P, MQ, T], bf16)
        v_sb = persist.tile([P, TK, HG * VW], bf16)
        oT_sb = persist.tile([P, MQ, T], bf16)
        v4d = v_sb.rearrange("p t (h c) -> p t h c", h=HG)
        nc.vector.memset(v4d[:, :, :, HD : HD + 1], 1.0)
        ident = const.tile([P, P], bf16)
        masks.make_identity(nc, ident)

        # ---- input DMAs: x on the SP queue, weights on the ACT queue so the
        # first q/k projections are not serialized behind the weight loads
        x_sb = xw.tile([P, KD, T], bf16)
        wq_sb = xw.tile([P, KD, DG], bf16)
        wk_sb = xw.tile([P, KD, DG], bf16)
        bq_sb = const.tile([P, MQ], f32)
        nc.sync.dma_start(out=bq_sb, in_=bq)
        bk_sb = const.tile([P, MQ], f32)
        nc.sync.dma_start(out=bk_sb, in_=bk)
        # wk leads the ACT queue (first PE op is its LDWEIGHTS), x is split
        # across both queues, wq follows the even x half on SP
        for ki in range(KD):
            nc.scalar.dma_start(out=wk_sb[:, ki], in_=wk[:, ki])
        for ki in range(0, KD, 2):
            nc.sync.dma_start(out=x_sb[:, ki], in_=xT[:, ki])
        for ki in range(1, KD, 2):
            nc.scalar.dma_start(out=x_sb[:, ki], in_=xT[:, ki])
        for ki in range(KD):
            nc.sync.dma_start(out=wq_sb[:, ki], in_=wq[:, ki])
        wv_sb = xw.tile([P, KD, DG], bf16)
        nc.scalar.dma_start(out=wv_sb, in_=wv)
        wo_sb = const.tile([P, MQ, D], bf16)
        nc.scalar.dma_start(out=wo_sb, in_=wo)

        # accumulate in x-arrival order (even k-tiles land first on SP)
        ki_order = list(range(0, KD, 2)) + list(range(1, KD, 2))

        def gen_qk(mt, n, which="kq"):
            """Generator: one T-chunk (n) of the q and/or k projection."""
            sel = {
                "k": ((wk_sb, bk_sb, k_sb),),
                "q": ((wq_sb, bq_sb, q_sb),),
                "kq": ((wk_sb, bk_sb, k_sb), (wq_sb, bq_sb, q_sb)),
            }[which]
            for w_sb, b_col, dst in sel:
                ps = qkvps.tile([P, TQC], f32, tag="qkv", name="ps_qkv")
                for idx, ki in enumerate(ki_order):
                    nc.tensor.matmul(
                        ps,
                        w_sb[:, ki, mt * P : (mt + 1) * P],
                        x_sb[:, ki, n * TQC : (n + 1) * TQC],
                        start=(idx == 0),
                        stop=(idx == KD - 1),
                    )
                    if idx == 3:
                        yield
                nc.vector.tensor_scalar_add(
                    dst[:, mt, n * TQC : (n + 1) * TQC], ps, b_col[:, mt : mt + 1]
                )
                yield

        def gen_v(t0, t1):
            """Generator: V projection for token tiles [t0, t1)."""
            for t in range(t0, t1):
                ps = qkvps.tile([P, DG], f32, tag="qkv", name="ps_v")
                for idx, ki in enumerate(ki_order):
                    nc.tensor.matmul(
                        ps,
                        x_sb[:, ki, t * P : (t + 1) * P],
                        wv_sb[:, ki, :],
                        start=(idx == 0),
                        stop=(idx == KD - 1),
                    )
                    if idx == 3:
                        yield
                nc.vector.tensor_copy(
                    v4d[:, t, :, 0:HD], ps.rearrange("p (h c) -> p h c", h=HG)
                )
                yield

        def gen_pvt(p, c, pts):
            """Generator: transposed PV + normalize + o-transpose for the
            window (p, c) whose probabilities live in pts."""
            tq0 = c * TQC
            o_nat = onp.tile([P, MQ, P], bf16, name="onat")
            for t in range(2):
                pv = pvtp.tile([P, 2, 2, VW], f32, tag="pv", name="pv")
                for j in range(2):
                    qs = (2 * t + j) * P
                    for i in range(2):
                        h = 2 * p + i
                        for half in range(2):
                            for tk in range(8 * half, 8 * half + 8):
                                nc.tensor.matmul(
                                    pv[:, j, i, :],
                                    pts[i][:, tk, qs : qs + P],
                                    v_sb[:, tk, h * VW : (h + 1) * VW],
                                    start=(tk == 0),
                                    stop=(tk == TK - 1),
                                )
                            yield
                rc = nrm.tile([P, 2, 2, 1], f32, name="rc")
                nc.vector.reciprocal(rc, pv[:, :, :, HD : HD + 1])
                for j in range(2):
                    for i in range(2):
                        nc.vector.tensor_scalar_mul(
                            o_nat[:, 2 * t + j, i * HD : (i + 1) * HD],
                            pv[:, j, i, 0:HD],
                            rc[:, j, i, :],
                        )
                yield
            tpv = pvtp.tile([P, MQ, P], bf16, tag="pv", name="tpv")
            for qt in range(MQ):
                nc.tensor.transpose(tpv[:, qt, :], o_nat[:, qt, :], ident)
            nc.vector.tensor_copy(
                oT_sb[:, p, tq0 : tq0 + TQC], tpv.rearrange("p a b -> p (a b)")
            )
            yield

        def gen_oproj(c):
            tq0 = c * TQC
            for j in range(D // P):
                ys = qkvps.tile([P, TQC], f32, tag="qkv", name="ys")
                for ki in range(MQ):
                    nc.tensor.matmul(
                        ys,
                        wo_sb[:, ki, j * P : (j + 1) * P],
                        oT_sb[:, ki, tq0 : tq0 + TQC],
                        start=(ki == 0),
                        stop=(ki == MQ - 1),
                    )
                yo = yop.tile([P, TQC], f32, name="yo")
                nc.vector.tensor_copy(yo, ys)
                nc.sync.dma_start(out=yT[:, j, tq0 : tq0 + TQC], in_=yo)
                yield

        def sc_group(pt, p, hb, tq0, g):
            scs = scps.tile([P, 2, TQC], f32, tag="sc", name="sc")
            for u in range(2):
                tk = 2 * g + u
                nc.tensor.matmul(
                    scs[:, u, :],
                    k_sb[hb : hb + HD, p, tk * P : (tk + 1) * P],
                    q_sb[hb : hb + HD, p, tq0 : tq0 + TQC],
                    start=True,
                    stop=True,
                )
            nc.scalar.activation(pt[:, 2 * g : 2 * g + 2, :], scs, AF.Exp, scale=SCALE)

        def drain(gens):
            while gens:
                try:
                    next(gens[0])
                    return True
                except StopIteration:
                    gens.pop(0)
            return False

        def window(p, c, pts, gens):
            tq0 = c * TQC
            for i in range(2):
                hb = i * HD
                for g in range(8):
                    sc_group(pts[i], p, hb, tq0, g)
                    for _ in range(2):
                        if not drain(gens):
                            break
            while drain(gens):
                pass

        # ---- schedule: pair-outer, chunk-inner; PV trails one window ----
        if reps > 1:
            loop_cm = tc.For_i(0, reps, 1)
            loop_cm.__enter__()

        for g_ in gen_qk(0, 0, "k"):
            pass
        for n in range(1, NC2):
            for g_ in gen_qk(0, n, "k"):
                pass
        for g_ in gen_qk(0, 0, "q"):
            pass

        prev = None
        for p in range(MQ):
            for c in range(NC2):
                pts = [
                    ptp.tile([P, TK, TQC], bf16, tag="pt", name="pt0"),
                    ptp.tile([P, TK, TQC], bf16, tag="pt", name="pt1"),
                ]
                gens = []
                if p == 0:
                    # V projection: finished by the end of window (0, 1); it
                    # precedes the trailing PV which reads all of v_sb
                    if c == 0:
                        gens.append(gen_v(0, 10))
                    elif c == 1:
                        gens.append(gen_v(10, TK))
                if prev is not None:
                    gens.append(gen_pvt(*prev))
                if p == 0:
                    if c < NC2 - 1:
                        gens.append(gen_qk(0, c + 1, "q"))
                if p < MQ - 1:
                    gens.append(gen_qk(p + 1, c, "kq"))
                if p == MQ - 1 and c > 0:
                    gens.append(gen_oproj(c - 1))
                window(p, c, pts, gens)
                prev = (p, c, pts)

        # epilogue: last window's PV + final output projection
        gens = [gen_pvt(*prev), gen_oproj(NC2 - 1)]
        while drain(gens):
            pass

        if reps > 1:
            loop_cm.__exit__(None, None, None)


def _build(debug=False, reps=1):
    import concourse.tile as tile
    from concourse import bacc, mybir

    dt = mybir.dt
    f32, bf16 = dt.float32, dt.bfloat16

    nc = bacc.Bacc("TRN2", target_bir_lowering=False, debug=False)
    # inputs are host-preswizzled into partition-major layouts so every DMA
    # descriptor is a fat contiguous run
    aps = {
        "xT": nc.dram_tensor("xT", [P, KD, T], bf16, kind="ExternalInput").ap(),
        "wq": nc.dram_tensor("wq", [P, KD, DG], bf16, kind="ExternalInput").ap(),
        "wk": nc.dram_tensor("wk", [P, KD, DG], bf16, kind="ExternalInput").ap(),
        "wv": nc.dram_tensor("wv", [P, KD, DG], bf16, kind="ExternalInput").ap(),
        "wo": nc.dram_tensor("wo", [P, MQ, D], bf16, kind="ExternalInput").ap(),
        "bq": nc.dram_tensor("bq", [P, MQ], f32, kind="ExternalInput").ap(),
        "bk": nc.dram_tensor("bk", [P, MQ], f32, kind="ExternalInput").ap(),
        "yT": nc.dram_tensor("yT", [P, D // P, T], f32, kind="ExternalOutput").ap(),
    }

    with tile.TileContext(nc) as tc:
        _emit(tc, aps, reps=reps)
    nc.compile()
    return nc


def _get_nc():
    if "nc" not in _CACHE:
        _CACHE["nc"] = _build()
    return _CACHE["nc"]


def _shard_inputs(x, Wq, bq, Wk, bk, Wv, bv, Wo, bo):
    import ml_dtypes

    bf16 = ml_dtypes.bfloat16
    f32 = np.float32

    def c(a, dtype):
        return np.ascontiguousarray(a).astype(dtype)

    def kp(a, kt):  # [kt*P, F] -> [P, kt, F] partition-major swizzle
        return a.reshape(kt, P, a.shape[-1]).transpose(1, 0, 2)

    in_maps = []
    for core in range(NCORES):
        b, g = core // 2, core % 2
        hs = g * DG
        in_maps.append(
            {
                "xT": c(kp(x[b].T, KD), bf16),
                "wq": c(kp(Wq[hs : hs + DG, :].T, KD), bf16),
                "wk": c(kp(Wk[hs : hs + DG, :].T, KD), bf16),
                "wv": c(kp(Wv[hs : hs + DG, :].T, KD), bf16),
                "wo": c(kp(Wo[:, hs : hs + DG].T, MQ), bf16),
                "bq": c(bq[hs : hs + DG].reshape(MQ, P).T, f32),
                "bk": c(bk[hs : hs + DG].reshape(MQ, P).T, f32),
            }
        )
    return in_maps


def _run(inputs, trace=False):
    from concourse import bass_utils

    nc = _get_nc()
    np_in = {k: np.asarray(v) for k, v in inputs.items()}
    in_maps = _shard_inputs(**np_in)
    res = bass_utils.run_bass_kernel_spmd(
        nc, in_maps, core_ids=list(range(NCORES)), trace=trace
    )
    # softmax rows sum to 1, so the V bias passes through attention as
    # exactly +bv; fold bv @ Wo.T (and bo) on the host.
    corr = np_in["bo"].astype(np.float32) + np_in["bv"].astype(
        np.float32
    ) @ np_in["Wo"].astype(np.float32).T
    y = np.empty((B, T, D), dtype=np.float32)
    for b in range(B):
        acc = res.results[2 * b]["yT"] + res.results[2 * b + 1]["yT"]  # [P, D/P, T]
        y[b] = acc.transpose(1, 0, 2).reshape(D, T).T + corr
    return y, res


def kernel(**inputs):
    y, _ = _run(inputs)
    return y


# revision 11
# speedup vs baseline: 1.1511x; 1.0120x over previous
"""Multi-head attention (B=4, T=2048, D=1024, H=16) on 8 Trainium2 NeuronCores.

Sharding: core = (batch, head-group): b = core // 2, g = core % 2.
Each core computes heads [g*8, g*8+8) of batch b:
  - Q/K projections into transposed layout qT/kT = W_g @ x_b.T  [512, 2048]
  - V projection in natural layout [2048, 512] plus a ones column per head
  - scores computed transposed: S.T tile = K_h @ Q_h.T on the PE; exp fused
    on ScalarE over two-bank PSUM groups (FD=1024), scale=1/sqrt(64),
    no max subtraction (logits ~N(0,1))
  - PV transposed: out[128q, 65] = pT-slice[128k,128q].T @ [V_h|1][128k,65]
    so the matmul streams only 65 columns per k-tile (cost model charges
    N=out-free-size) and the softmax row-sum lands lane-wise in column 64
  - normalize is a per-partition reciprocal + tensor_scalar_mul on DVE
  - o is transposed back with PE transpose-matmuls for the output projection
  - partial output projection yT_g = Wo[:, g].T-contraction  [1024, 2048]
Host: y[b] = (yT_part[2b] + yT_part[2b+1]).T + bo + bv @ Wo.T
(softmax rows sum to one, so the V bias contributes exactly bv @ Wo.T).

The emission interleaves PE filler work (PV of the previous window, Q/K
prefetch, V projection, output projection) between score/exp groups so the
PE never waits on ScalarE's exp stream.

Self-contained: hardcodes all shapes; requires only concourse (bass) + numpy.
"""

import numpy as np

B, T, D = 4, 2048, 1024
H, HD = 16, 64
HG, DG = 8, 512          # heads / feature columns per core
NCORES = 8
P = 128
KD = D // P              # 8  k-tiles over model dim
MQ = DG // P             # 4  partition tiles of qT/kT/oT (one per head pair)
TK = T // P              # 16 key tiles
TQC = 512                # query-chunk (= one fp32 PSUM bank)
NC2 = T // TQC           # 4  query chunks
VW = HD + 1              # V columns per head incl. ones column
SCALE = 0.125            # 1/sqrt(HD)

_CACHE: dict = {}


def _emit(tc, aps, reps=1):
    import concourse.bass as bass  # noqa: F401
    from concourse import masks, mybir

    nc = tc.nc
    dt = mybir.dt
    f32, bf16 = dt.float32, dt.bfloat16
    AF = mybir.ActivationFunctionType
    xT, wq, wk, wv, wo, bq, bk, yT = (
        aps["xT"], aps["wq"], aps["wk"], aps["wv"], aps["wo"],
        aps["bq"], aps["bk"], aps["yT"],
    )

    from contextlib import ExitStack

    with ExitStack() as ctx:
        const = ctx.enter_context(tc.tile_pool(name="const", bufs=1))
        persist = ctx.enter_context(tc.tile_pool(name="persist", bufs=1))
        xw = ctx.enter_context(tc.tile_pool(name="xw", bufs=1))
        ptp = ctx.enter_context(tc.tile_pool(name="ptp", bufs=4))
        onp = ctx.enter_context(tc.tile_pool(name="onp", bufs=2))
        yop = ctx.enter_context(tc.tile_pool(name="yop", bufs=3))
        nrm = ctx.enter_context(tc.tile_pool(name="nrm", bufs=3))
        scps = ctx.enter_context(tc.tile_pool(name="scps", bufs=2, space="PSUM"))
        qkvps = ctx.enter_context(tc.tile_pool(name="qkvps", bufs=2, space="PSUM"))
        pvtp = ctx.enter_context(tc.tile_pool(name="pvtp", bufs=2, space="PSUM"))

        # ---- persistent SBUF ----
        q_sb = persist.tile([P, MQ, T], bf16)
        k_sb = persist.tile([P, MQ, T], bf16)
        v_sb = persist.tile([P, TK, HG * VW], bf16)
        oT_sb = persist.tile([P, MQ, T], bf16)
        v4d = v_sb.rearrange("p t (h c) -> p t h c", h=HG)
        nc.vector.memset(v4d[:, :, :, HD : HD + 1], 1.0)
        ident = const.tile([P, P], bf16)
        masks.make_identity(nc, ident)

        # ---- input DMAs, ordered for earliest first score group: wk leads,
        # then x arrives chunk-major in [P, 512] pieces (evens on SP, odds on
        # the ACT queue) so k(mt0, n) and the first score groups start as
        # chunk n lands instead of after the full x transfer
        x_sb = xw.tile([P, KD, T], bf16)
        wq_sb = xw.tile([P, KD, DG], bf16)
        wk_sb = xw.tile([P, KD, DG], bf16)
        bq_sb = const.tile([P, MQ], f32)
        nc.sync.dma_start(out=bq_sb, in_=bq)
        bk_sb = const.tile([P, MQ], f32)
        nc.sync.dma_start(out=bk_sb, in_=bk)
        for ki in range(KD):
            nc.scalar.dma_start(out=wk_sb[:, ki], in_=wk[:, ki])
        for tc in range(NC2):
            s = slice(tc * TQC, (tc + 1) * TQC)
            for ki in range(0, KD, 2):
                nc.sync.dma_start(out=x_sb[:, ki, s], in_=xT[:, ki, s])
            for ki in range(1, KD, 2):
                nc.scalar.dma_start(out=x_sb[:, ki, s], in_=xT[:, ki, s])
            if tc == 0:
                for ki in range(KD):
                    nc.sync.dma_start(out=wq_sb[:, ki], in_=wq[:, ki])
        wv_sb = xw.tile([P, KD, DG], bf16)
        nc.scalar.dma_start(out=wv_sb, in_=wv)
        wo_sb = const.tile([P, MQ, D], bf16)
        nc.scalar.dma_start(out=wo_sb, in_=wo)

        # accumulate in x-arrival order (even k-tiles land first on SP)
        ki_order = list(range(0, KD, 2)) + list(range(1, KD, 2))

        def gen_qk(mt, n, which="kq"):
            """Generator: one T-chunk (n) of the q and/or k projection."""
            sel = {
                "k": ((wk_sb, bk_sb, k_sb),),
                "q": ((wq_sb, bq_sb, q_sb),),
                "kq": ((wk_sb, bk_sb, k_sb), (wq_sb, bq_sb, q_sb)),
            }[which]
            for w_sb, b_col, dst in sel:
                ps = qkvps.tile([P, TQC], f32, tag="qkv", name="ps_qkv")
                for idx, ki in enumerate(ki_order):
                    nc.tensor.matmul(
                        ps,
                        w_sb[:, ki, mt * P : (mt + 1) * P],
                        x_sb[:, ki, n * TQC : (n + 1) * TQC],
                        start=(idx == 0),
                        stop=(idx == KD - 1),
                    )
                    if idx == 3:
                        yield
                nc.vector.tensor_scalar_add(
                    dst[:, mt, n * TQC : (n + 1) * TQC], ps, b_col[:, mt : mt + 1]
                )
                yield

        def gen_v(vp, t0, t1):
            """Generator: V projection of head pair vp for token tiles
            [t0, t1) — split by pair so it spreads across rows 0-2."""
            for t in range(t0, t1):
                ps = qkvps.tile([P, P], f32, tag="qkv", name="ps_v")
                for idx, ki in enumerate(ki_order):
                    nc.tensor.matmul(
                        ps,
                        x_sb[:, ki, t * P : (t + 1) * P],
                        wv_sb[:, ki, vp * P : (vp + 1) * P],
                        start=(idx == 0),
                        stop=(idx == KD - 1),
                    )
                nc.vector.tensor_copy(
                    v4d[:, t, 2 * vp : 2 * vp + 2, 0:HD],
                    ps.rearrange("p (h c) -> p h c", h=2),
                )
                yield

        def gen_pvt(p, c, pts):
            """Generator: transposed PV + normalize + o-transpose for the
            window (p, c) whose probabilities live in pts. Head-outer order:
            steps that need the latest exp groups of (p, c) are pulled last."""
            tq0 = c * TQC
            o_nat = onp.tile([P, MQ, P], bf16, name="onat")
            pv = [
                pvtp.tile([P, 2, 2, VW], f32, tag="pv", name="pv0"),
                pvtp.tile([P, 2, 2, VW], f32, tag="pv", name="pv1"),
            ]
            # head-outer so the steps needing head 1's exps (the last half of
            # the previous window's exp stream) are pulled last; a matmul
            # start lazily zeroes its whole 2KB psum bank, so each (t, j, i)
            # accumulation group runs start-to-stop before the next opens
            for i in range(2):
                h = 2 * p + i
                for t in range(2):
                    for j in range(2):
                        qs = (2 * t + j) * P
                        for half in range(2):
                            for tk in range(8 * half, 8 * half + 8):
                                nc.tensor.matmul(
                                    pv[t][:, j, i, :],
                                    pts[i][:, tk, qs : qs + P],
                                    v_sb[:, tk, h * VW : (h + 1) * VW],
                                    start=(tk == 0),
                                    stop=(tk == TK - 1),
                                )
                            yield
            for t in range(2):
                rc = nrm.tile([P, 2, 2, 1], f32, name="rc")
                nc.vector.reciprocal(rc, pv[t][:, :, :, HD : HD + 1])
                for j in range(2):
                    for i in range(2):
                        nc.vector.tensor_scalar_mul(
                            o_nat[:, 2 * t + j, i * HD : (i + 1) * HD],
                            pv[t][:, j, i, 0:HD],
                            rc[:, j, i, :],
                        )
                yield
            tpv = pvtp.tile([P, MQ, P], bf16, tag="pv", name="tpv")
            for qt in range(MQ):
                nc.tensor.transpose(tpv[:, qt, :], o_nat[:, qt, :], ident)
            nc.vector.tensor_copy(
                oT_sb[:, p, tq0 : tq0 + TQC], tpv.rearrange("p a b -> p (a b)")
            )
            yield

        def gen_oproj(c, pool_copy=False):
            tq0 = c * TQC
            for j in range(D // P):
                ys = qkvps.tile([P, TQC], f32, tag="qkv", name="ys")
                for ki in range(MQ):
                    nc.tensor.matmul(
                        ys,
                        wo_sb[:, ki, j * P : (j + 1) * P],
                        oT_sb[:, ki, tq0 : tq0 + TQC],
                        start=(ki == 0),
                        stop=(ki == MQ - 1),
                    )
                yo = yop.tile([P, TQC], f32, name="yo")
                # the epilogue's copies alternate onto the idle Pool engine so
                # the final drain is not serialized behind one engine's queue
                eng = nc.gpsimd if pool_copy and j % 2 else nc.vector
                eng.tensor_copy(yo, ys)
                nc.sync.dma_start(out=yT[:, j, tq0 : tq0 + TQC], in_=yo)
                yield

        def gen_spacer(n):
            for _ in range(n):
                yield

        def sc_group(pt, p, hb, tq0, g):
            scs = scps.tile([P, 2, TQC], f32, tag="sc", name="sc")
            for u in range(2):
                tk = 2 * g + u
                nc.tensor.matmul(
                    scs[:, u, :],
                    k_sb[hb : hb + HD, p, tk * P : (tk + 1) * P],
                    q_sb[hb : hb + HD, p, tq0 : tq0 + TQC],
                    start=True,
                    stop=True,
                )
            nc.scalar.activation(pt[:, 2 * g : 2 * g + 2, :], scs, AF.Exp, scale=SCALE)

        def drain(gens):
            while gens:
                try:
                    next(gens[0])
                    return True
                except StopIteration:
                    gens.pop(0)
            return False

        def window(p, c, pts, gens):
            tq0 = c * TQC
            for i in range(2):
                hb = i * HD
                for g in range(8):
                    sc_group(pts[i], p, hb, tq0, g)
                    for _ in range(2):
                        if not drain(gens):
                            break
            while drain(gens):
                pass

        # ---- schedule: pair-outer, chunk-inner; PV trails one window;
        # V-projection of pair p+1 spreads across row p to even out the
        # per-window PE load against ScalarE's fixed 16-exp stream ----
        if reps > 1:
            loop_cm = tc.For_i(0, reps, 1)
            loop_cm.__enter__()

        for g_ in gen_qk(0, 0, "k"):
            pass
        for g_ in gen_qk(0, 0, "q"):
            pass

        prev = None
        for p in range(MQ):
            for c in range(NC2):
                pts = [
                    ptp.tile([P, TK, TQC], bf16, tag="pt", name="pt0"),
                    ptp.tile([P, TK, TQC], bf16, tag="pt", name="pt1"),
                ]
                gens = []
                if p == 0 and c == 0:
                    for n in range(1, NC2):
                        gens.append(gen_qk(0, n, "k"))
                    gens.append(gen_v(0, 0, TK))
                elif p == 0:
                    t0 = [0, 0, 6, 11][c]
                    gens.append(gen_v(1, t0, [0, 6, 11, TK][c]))
                elif p < MQ - 1:
                    gens.append(gen_v(p + 1, 4 * c, 4 * c + 4))
                if prev is not None:
                    gens.append(gen_pvt(*prev))
                if p == 0 and c < NC2 - 1:
                    gens.append(gen_qk(0, c + 1, "q"))
                if p < MQ - 1:
                    gens.append(gen_qk(p + 1, c, "kq"))
                if p == MQ - 1 and c > 0:
                    gens.append(gen_spacer(2))
                    gens.append(gen_oproj(c - 1))
                window(p, c, pts, gens)
                prev = (p, c, pts)

        # epilogue: last window's PV + final output projection
        gens = [gen_pvt(*prev), gen_spacer(1), gen_oproj(NC2 - 1)]
        while drain(gens):
            pass

        if reps > 1:
            loop_cm.__exit__(None, None, None)


def _build(debug=False, reps=1):
    import concourse.tile as tile
    from concourse import bacc, mybir

    dt = mybir.dt
    f32, bf16 = dt.float32, dt.bfloat16

    nc = bacc.Bacc("TRN2", target_bir_lowering=False, debug=False)
    # inputs are host-preswizzled into partition-major layouts so every DMA
    # descriptor is a fat contiguous run
    aps = {
        "xT": nc.dram_tensor("xT", [P, KD, T], bf16, kind="ExternalInput").ap(),
        "wq": nc.dram_tensor("wq", [P, KD, DG], bf16, kind="ExternalInput").ap(),
        "wk": nc.dram_tensor("wk", [P, KD, DG], bf16, kind="ExternalInput").ap(),
        "wv": nc.dram_tensor("wv", [P, KD, DG], bf16, kind="ExternalInput").ap(),
        "wo": nc.dram_tensor("wo", [P, MQ, D], bf16, kind="ExternalInput").ap(),
        "bq": nc.dram_tensor("bq", [P, MQ], f32, kind="ExternalInput").ap(),
        "bk": nc.dram_tensor("bk", [P, MQ], f32, kind="ExternalInput").ap(),
        "yT": nc.dram_tensor("yT", [P, D // P, T], f32, kind="ExternalOutput").ap(),
    }

    with tile.TileContext(nc) as tc:
        _emit(tc, aps, reps=reps)
    nc.compile()
    return nc


def _get_nc():
    if "nc" not in _CACHE:
        _CACHE["nc"] = _build()
    return _CACHE["nc"]


def _shard_inputs(x, Wq, bq, Wk, bk, Wv, bv, Wo, bo):
    import ml_dtypes

    bf16 = ml_dtypes.bfloat16
    f32 = np.float32

    def c(a, dtype):
        return np.ascontiguousarray(a).astype(dtype)

    def kp(a, kt):  # [kt*P, F] -> [P, kt, F] partition-major swizzle
        return a.reshape(kt, P, a.shape[-1]).transpose(1, 0, 2)

    in_maps = []
    for core in range(NCORES):
        b, g = core // 2, core % 2
        hs = g * DG
        in_maps.append(
            {
                "xT": c(kp(x[b].T, KD), bf16),
                "wq": c(kp(Wq[hs : hs + DG, :].T, KD), bf16),
                "wk": c(kp(Wk[hs : hs + DG, :].T, KD), bf16),
                "wv": c(kp(Wv[hs : hs + DG, :].T, KD), bf16),
                "wo": c(kp(Wo[:, hs : hs + DG].T, MQ), bf16),
                "bq": c(bq[hs : hs + DG].reshape(MQ, P).T, f32),
                "bk": c(bk[hs : hs + DG].reshape(MQ, P).T, f32),
            }
        )
    return in_maps


def _run(inputs, trace=False):
    from concourse import bass_utils

    nc = _get_nc()
    np_in = {k: np.asarray(v) for k, v in inputs.items()}
    in_maps = _shard_inputs(**np_in)
    res = bass_utils.run_bass_kernel_spmd(
        nc, in_maps, core_ids=list(range(NCORES)), trace=trace
    )
    # softmax rows sum to 1, so the V bias passes through attention as
    # exactly +bv; fold bv @ Wo.T (and bo) on the host.
    corr = np_in["bo"].astype(np.float32) + np_in["bv"].astype(
        np.float32
    ) @ np_in["Wo"].astype(np.float32).T
    y = np.empty((B, T, D), dtype=np.float32)
    for b in range(B):
        acc = res.results[2 * b]["yT"] + res.results[2 * b + 1]["yT"]  # [P, D/P, T]
        y[b] = acc.transpose(1, 0, 2).reshape(D, T).T + corr
    return y, res


def kernel(**inputs):
    y, _ = _run(inputs)
    return y


# revision 17
# speedup vs baseline: 1.2446x; 1.0812x over previous
"""Multi-head attention (B=4, T=2048, D=1024, H=16) on 8 Trainium2 NeuronCores.

Sharding: core = (batch, head-group): b = core // 2, g = core % 2.
Each core computes heads [g*8, g*8+8) of batch b:
  - Q/K projections into transposed layout qT/kT = W_g @ x_b.T  [512, 2048]
  - V projection in natural layout [2048, 512] plus a ones column per head
  - scores computed transposed: S.T tile = K_h @ Q_h.T on the PE; exp fused
    on ScalarE over two-bank PSUM groups (FD=1024), scale=1/sqrt(64),
    no max subtraction (logits ~N(0,1))
  - PV transposed: out[128q, 65] = pT-slice[128k,128q].T @ [V_h|1][128k,65]
    so the matmul streams only 65 columns per k-tile (cost model charges
    N=out-free-size) and the softmax row-sum lands lane-wise in column 64
  - normalize is a per-partition reciprocal + tensor_scalar_mul on DVE
  - o is transposed back with PE transpose-matmuls for the output projection
  - partial output projection yT_g = Wo[:, g].T-contraction  [1024, 2048]
Host: y[b] = (yT_part[2b] + yT_part[2b+1]).T + bo + bv @ Wo.T
(softmax rows sum to one, so the V bias contributes exactly bv @ Wo.T).

The emission interleaves PE filler work (PV of the previous window, Q/K
prefetch, V projection, output projection) between score/exp groups so the
PE never waits on ScalarE's exp stream.

Self-contained: hardcodes all shapes; requires only concourse (bass) + numpy.
"""

import numpy as np

B, T, D = 4, 2048, 1024
H, HD = 16, 64
HG, DG = 8, 512          # heads / feature columns per core
NCORES = 8
P = 128
KD = D // P              # 8  k-tiles over model dim
MQ = DG // P             # 4  partition tiles of qT/kT/oT (one per head pair)
TK = T // P              # 16 key tiles
TQC = 512                # query-chunk (= one fp32 PSUM bank)
NC2 = T // TQC           # 4  query chunks
VW = HD + 1              # V columns per head incl. ones column
SCALE = 0.125            # 1/sqrt(HD)

_CACHE: dict = {}


def _emit(tc, aps, reps=1):
    import concourse.bass as bass  # noqa: F401
    from concourse import masks, mybir

    nc = tc.nc
    dt = mybir.dt
    f32, bf16 = dt.float32, dt.bfloat16
    AF = mybir.ActivationFunctionType
    xT, wq, wk, wv, wo, bq, bk, yT = (
        aps["xT"], aps["wq"], aps["wk"], aps["wv"], aps["wo"],
        aps["bq"], aps["bk"], aps["yT"],
    )

    from contextlib import ExitStack

    with ExitStack() as ctx:
        const = ctx.enter_context(tc.tile_pool(name="const", bufs=1))
        persist = ctx.enter_context(tc.tile_pool(name="persist", bufs=1))
        xw = ctx.enter_context(tc.tile_pool(name="xw", bufs=1))
        ptp = ctx.enter_context(tc.tile_pool(name="ptp", bufs=4))
        onp = ctx.enter_context(tc.tile_pool(name="onp", bufs=2))
        yop = ctx.enter_context(tc.tile_pool(name="yop", bufs=3))
        nrm = ctx.enter_context(tc.tile_pool(name="nrm", bufs=3))
        scps = ctx.enter_context(tc.tile_pool(name="scps", bufs=2, space="PSUM"))
        qkvps = ctx.enter_context(tc.tile_pool(name="qkvps", bufs=2, space="PSUM"))
        pvtp = ctx.enter_context(tc.tile_pool(name="pvtp", bufs=2, space="PSUM"))

        # ---- persistent SBUF ----
        q_sb = persist.tile([P, MQ, T], bf16)
        k_sb = persist.tile([P, MQ, T], bf16)
        v_sb = persist.tile([P, TK, HG * VW], bf16)
        oT_sb = persist.tile([P, MQ, T], bf16)
        v4d = v_sb.rearrange("p t (h c) -> p t h c", h=HG)
        nc.vector.memset(v4d[:, :, :, HD : HD + 1], 1.0)
        ident = const.tile([P, P], bf16)
        masks.make_identity(nc, ident)

        # ---- input DMAs. The shared HWDGE device costs ~625 ns per DMA, so
        # transfers are few and big, emitted on one queue in exactly the
        # order the prologue consumes them: the mt0 slices of wk/wq and x
        # chunk 0 arrive first (split in ki halves so the first projection
        # groups overlap the transfer), then the later x chunks, then the
        # weights the fillers need later.
        x_sb = xw.tile([P, KD, T], bf16)
        wq_sb = xw.tile([P, KD, DG], bf16)
        wk_sb = xw.tile([P, KD, DG], bf16)
        bq_sb = const.tile([P, MQ], f32)
        bk_sb = const.tile([P, MQ], f32)
        wv_sb = xw.tile([P, KD, DG], bf16)
        wo_sb = const.tile([P, MQ, D], bf16)
        KH = KD // 2
        c0 = slice(0, TQC)
        nc.sync.dma_start(out=wk_sb[:, 0:KH, 0:P], in_=wk[:, 0:KH, 0:P])
        nc.sync.dma_start(out=x_sb[:, 0:KH, c0], in_=xT[:, 0:KH, c0])
        nc.sync.dma_start(out=wk_sb[:, KH:KD, 0:P], in_=wk[:, KH:KD, 0:P])
        nc.sync.dma_start(out=x_sb[:, KH:KD, c0], in_=xT[:, KH:KD, c0])
        nc.sync.dma_start(out=bq_sb, in_=bq)
        nc.sync.dma_start(out=bk_sb, in_=bk)
        nc.sync.dma_start(out=wq_sb[:, 0:KH, 0:P], in_=wq[:, 0:KH, 0:P])
        nc.sync.dma_start(out=wq_sb[:, KH:KD, 0:P], in_=wq[:, KH:KD, 0:P])
        nc.sync.dma_start(out=wv_sb, in_=wv)
        for tc in (1, 2, 3):
            s = slice(tc * TQC, (tc + 1) * TQC)
            nc.sync.dma_start(out=x_sb[:, :, s], in_=xT[:, :, s])
        nc.sync.dma_start(out=wk_sb[:, :, P:DG], in_=wk[:, :, P:DG])
        nc.sync.dma_start(out=wq_sb[:, :, P:DG], in_=wq[:, :, P:DG])
        nc.sync.dma_start(out=wo_sb, in_=wo)

        # accumulation order matches the ki-half arrival of the first pieces
        ki_order = list(range(KD))

        def gen_qk(mt, n, which="kq"):
            """Generator: one T-chunk (n) of the q and/or k projection."""
            sel = {
                "k": ((wk_sb, bk_sb, k_sb),),
                "q": ((wq_sb, bq_sb, q_sb),),
                "kq": ((wk_sb, bk_sb, k_sb), (wq_sb, bq_sb, q_sb)),
            }[which]
            for w_sb, b_col, dst in sel:
                ps = qkvps.tile([P, TQC], f32, tag="qkv", name="ps_qkv")
                for idx, ki in enumerate(ki_order):
                    nc.tensor.matmul(
                        ps,
                        w_sb[:, ki, mt * P : (mt + 1) * P],
                        x_sb[:, ki, n * TQC : (n + 1) * TQC],
                        start=(idx == 0),
                        stop=(idx == KD - 1),
                    )
                    if idx == 3:
                        yield
                nc.vector.tensor_scalar_add(
                    dst[:, mt, n * TQC : (n + 1) * TQC], ps, b_col[:, mt : mt + 1]
                )
                yield

        def gen_v(vp, t0, t1):
            """Generator: V projection of head pair vp for token tiles
            [t0, t1) — split by pair so it spreads across rows 0-2."""
            for t in range(t0, t1):
                ps = qkvps.tile([P, P], f32, tag="qkv", name="ps_v")
                for idx, ki in enumerate(ki_order):
                    nc.tensor.matmul(
                        ps,
                        x_sb[:, ki, t * P : (t + 1) * P],
                        wv_sb[:, ki, vp * P : (vp + 1) * P],
                        start=(idx == 0),
                        stop=(idx == KD - 1),
                    )
                nc.vector.tensor_copy(
                    v4d[:, t, 2 * vp : 2 * vp + 2, 0:HD],
                    ps.rearrange("p (h c) -> p h c", h=2),
                )
                yield

        def gen_pvt(p, c, pts):
            """Generator: transposed PV + normalize + o-transpose for the
            window (p, c) whose probabilities live in pts. Head-outer order:
            steps that need the latest exp groups of (p, c) are pulled last."""
            tq0 = c * TQC
            o_nat = onp.tile([P, MQ, P], bf16, name="onat")
            pv = [
                pvtp.tile([P, 2, 2, VW], f32, tag="pv", name="pv0"),
                pvtp.tile([P, 2, 2, VW], f32, tag="pv", name="pv1"),
            ]
            # head-outer so the steps needing head 1's exps (the last half of
            # the previous window's exp stream) are pulled last; a matmul
            # start lazily zeroes its whole 2KB psum bank, so each (t, j, i)
            # accumulation group runs start-to-stop before the next opens
            for i in range(2):
                h = 2 * p + i
                for t in range(2):
                    for j in range(2):
                        qs = (2 * t + j) * P
                        for half in range(2):
                            for tk in range(8 * half, 8 * half + 8):
                                nc.tensor.matmul(
                                    pv[t][:, j, i, :],
                                    pts[i][:, tk, qs : qs + P],
                                    v_sb[:, tk, h * VW : (h + 1) * VW],
                                    start=(tk == 0),
                                    stop=(tk == TK - 1),
                                )
                            yield
            for t in range(2):
                rc = nrm.tile([P, 2, 2, 1], f32, name="rc")
                nc.vector.reciprocal(rc, pv[t][:, :, :, HD : HD + 1])
                for j in range(2):
                    for i in range(2):
                        nc.vector.tensor_scalar_mul(
                            o_nat[:, 2 * t + j, i * HD : (i + 1) * HD],
                            pv[t][:, j, i, 0:HD],
                            rc[:, j, i, :],
                        )
                yield
            tpv = pvtp.tile([P, MQ, P], bf16, tag="pv", name="tpv")
            for qt in range(MQ):
                nc.tensor.transpose(tpv[:, qt, :], o_nat[:, qt, :], ident)
            nc.vector.tensor_copy(
                oT_sb[:, p, tq0 : tq0 + TQC], tpv.rearrange("p a b -> p (a b)")
            )
            yield

        def gen_oproj(c, pool_copy=False):
            tq0 = c * TQC
            for j in range(D // P):
                ys = qkvps.tile([P, TQC], f32, tag="qkv", name="ys")
                for ki in range(MQ):
                    nc.tensor.matmul(
                        ys,
                        wo_sb[:, ki, j * P : (j + 1) * P],
                        oT_sb[:, ki, tq0 : tq0 + TQC],
                        start=(ki == 0),
                        stop=(ki == MQ - 1),
                    )
                yo = yop.tile([P, TQC], bf16, name="yo")
                # the epilogue's copies alternate onto the idle Pool engine so
                # the final drain is not serialized behind one engine's queue
                eng = nc.gpsimd if pool_copy and j % 2 else nc.vector
                eng.tensor_copy(yo, ys)
                nc.sync.dma_start(out=yT[:, j, tq0 : tq0 + TQC], in_=yo)
                yield

        def gen_spacer(n):
            for _ in range(n):
                yield

        def sc_group(pt, p, hb, tq0, g):
            scs = scps.tile([P, 2, TQC], f32, tag="sc", name="sc")
            for u in range(2):
                tk = 2 * g + u
                nc.tensor.matmul(
                    scs[:, u, :],
                    k_sb[hb : hb + HD, p, tk * P : (tk + 1) * P],
                    q_sb[hb : hb + HD, p, tq0 : tq0 + TQC],
                    start=True,
                    stop=True,
                )
            nc.scalar.activation(pt[:, 2 * g : 2 * g + 2, :], scs, AF.Exp, scale=SCALE)

        def drain(gens):
            while gens:
                try:
                    next(gens[0])
                    return True
                except StopIteration:
                    gens.pop(0)
            return False

        def window(p, c, pts, gens, interleave_heads=False):
            # head-major keeps each head's exps contiguous so the trailing PV
            # (head-outer) never waits; window (0,0) instead paces key-tiles
            # slowest (head-interleaved) so the k(mt0, n) fillers stay ahead
            # of the score groups that read them
            tq0 = c * TQC
            if interleave_heads:
                order = [(g // 2, g % 2) for g in range(16)]
            else:
                order = [(g % 8, g // 8) for g in range(16)]
            for g, i in order:
                sc_group(pts[i], p, i * HD, tq0, g)
                for _ in range(2):
                    if not drain(gens):
                        break
            while drain(gens):
                pass

        # ---- schedule: pair-outer, chunk-inner; PV trails one window;
        # V-projection of pair p+1 spreads across row p to even out the
        # per-window PE load against ScalarE's fixed 16-exp stream ----
        if reps > 1:
            loop_cm = tc.For_i(0, reps, 1)
            loop_cm.__enter__()

        for g_ in gen_qk(0, 0, "k"):
            pass
        for g_ in gen_qk(0, 0, "q"):
            pass

        prev = None
        for p in range(MQ):
            for c in range(NC2):
                pts = [
                    ptp.tile([P, TK, TQC], bf16, tag="pt", name="pt0"),
                    ptp.tile([P, TK, TQC], bf16, tag="pt", name="pt1"),
                ]
                gens = []
                if p == 0 and c == 0:
                    # interleaved by DMA arrival: V token tiles of chunk n
                    # become ready together with k(mt0, n+1)'s x chunk
                    for n in range(1, NC2):
                        gens.append(gen_v(0, 4 * (n - 1), 4 * n))
                        gens.append(gen_qk(0, n, "k"))
                    gens.append(gen_v(0, 12, TK))
                elif p == 0:
                    t0 = [0, 0, 6, 11][c]
                    gens.append(gen_v(1, t0, [0, 6, 11, TK][c]))
                elif p < MQ - 1:
                    gens.append(gen_v(p + 1, 4 * c, 4 * c + 4))
                if prev is not None:
                    gens.append(gen_pvt(*prev))
                if p == 0 and c < NC2 - 1:
                    gens.append(gen_qk(0, c + 1, "q"))
                if p < MQ - 1:
                    gens.append(gen_qk(p + 1, c, "kq"))
                if p == MQ - 1 and c > 0:
                    gens.append(gen_spacer(2))
                    gens.append(gen_oproj(c - 1))
                window(p, c, pts, gens, interleave_heads=(p == 0 and c == 0))
                prev = (p, c, pts)

        # epilogue: last window's PV + final output projection
        gens = [gen_pvt(*prev), gen_spacer(1), gen_oproj(NC2 - 1)]
        while drain(gens):
            pass

        if reps > 1:
            loop_cm.__exit__(None, None, None)


def _build(debug=False, reps=1):
    import concourse.tile as tile
    from concourse import bacc, mybir

    dt = mybir.dt
    f32, bf16 = dt.float32, dt.bfloat16

    nc = bacc.Bacc("TRN2", target_bir_lowering=False, debug=False)
    # inputs are host-preswizzled into partition-major layouts so every DMA
    # descriptor is a fat contiguous run
    aps = {
        "xT": nc.dram_tensor("xT", [P, KD, T], bf16, kind="ExternalInput").ap(),
        "wq": nc.dram_tensor("wq", [P, KD, DG], bf16, kind="ExternalInput").ap(),
        "wk": nc.dram_tensor("wk", [P, KD, DG], bf16, kind="ExternalInput").ap(),
        "wv": nc.dram_tensor("wv", [P, KD, DG], bf16, kind="ExternalInput").ap(),
        "wo": nc.dram_tensor("wo", [P, MQ, D], bf16, kind="ExternalInput").ap(),
        "bq": nc.dram_tensor("bq", [P, MQ], f32, kind="ExternalInput").ap(),
        "bk": nc.dram_tensor("bk", [P, MQ], f32, kind="ExternalInput").ap(),
        "yT": nc.dram_tensor("yT", [P, D // P, T], bf16, kind="ExternalOutput").ap(),
    }

    with tile.TileContext(nc) as tc:
        _emit(tc, aps, reps=reps)
    nc.compile()
    return nc


def _get_nc():
    if "nc" not in _CACHE:
        _CACHE["nc"] = _build()
    return _CACHE["nc"]


def _shard_inputs(x, Wq, bq, Wk, bk, Wv, bv, Wo, bo):
    import ml_dtypes

    bf16 = ml_dtypes.bfloat16
    f32 = np.float32

    def c(a, dtype):
        return np.ascontiguousarray(a).astype(dtype)

    def kp(a, kt):  # [kt*P, F] -> [P, kt, F] partition-major swizzle
        return a.reshape(kt, P, a.shape[-1]).transpose(1, 0, 2)

    in_maps = []
    for core in range(NCORES):
        b, g = core // 2, core % 2
        hs = g * DG
        in_maps.append(
            {
                "xT": c(kp(x[b].T, KD), bf16),
                "wq": c(kp(Wq[hs : hs + DG, :].T, KD), bf16),
                "wk": c(kp(Wk[hs : hs + DG, :].T, KD), bf16),
                "wv": c(kp(Wv[hs : hs + DG, :].T, KD), bf16),
                "wo": c(kp(Wo[:, hs : hs + DG].T, MQ), bf16),
                "bq": c(bq[hs : hs + DG].reshape(MQ, P).T, f32),
                "bk": c(bk[hs : hs + DG].reshape(MQ, P).T, f32),
            }
        )
    return in_maps


def _run(inputs, trace=False):
    from concourse import bass_utils

    nc = _get_nc()
    np_in = {k: np.asarray(v) for k, v in inputs.items()}
    in_maps = _shard_inputs(**np_in)
    res = bass_utils.run_bass_kernel_spmd(
        nc, in_maps, core_ids=list(range(NCORES)), trace=trace
    )
    # softmax rows sum to 1, so the V bias passes through attention as
    # exactly +bv; fold bv @ Wo.T (and bo) on the host.
    corr = np_in["bo"].astype(np.float32) + np_in["bv"].astype(
        np.float32
    ) @ np_in["Wo"].astype(np.float32).T
    y = np.empty((B, T, D), dtype=np.float32)
    for b in range(B):
        acc = res.results[2 * b]["yT"].astype(np.float32) + res.results[
            2 * b + 1
        ]["yT"].astype(np.float32)  # [P, D/P, T]
        y[b] = acc.transpose(1, 0, 2).reshape(D, T).T + corr
    return y, res


def kernel(**inputs):
    y, _ = _run(inputs)
    return y
